# revision 8
# baseline (speedup 1.0000x reference)
"""Trainium2 kernel for the sobel-perception CNN cell (separable form).

Computation (per pixel, circular 3x3 stencil):
    perc = [sobel_x * x, sobel_y * x, x]            # 48 channels
    hidden = relu(W1 @ perc + b1)                   # 128 channels
    out    = W2 @ hidden + b2                       # 16 channels

Key transforms:
  * The sobel fields gx, gy are linear in x, so the host bakes them into
    the input slab (z = [gx, gy, x], 48 channels), the same way the
    baseline baked halos and shifted window copies host-side.  The
    device conv becomes a pure 1x1: hidden = relu(W1 @ z + b1).
  * Row pairing: even rows of a group live at SBUF partitions 0-47, odd
    rows at 64-111.  The two K=48 matmuls run CONCURRENTLY on disjoint
    PE row-strips via tile_position (0,0)/(64,0): 512 stream cycles per
    2 rows per 512-px chunk.
  * mm2 (K=128 -> M=16) packs 4 chunks into 4 concurrent 32-wide PE
    column groups writing one PSUM bank.
  * Evacuation: DVE does row-even hidden as ONE [128,1024] 2-bank fused
    tensor_scalar (bias+relu), Scalar does row-odd as one [128,1024]
    ACT.  mm2 collects alternate engines.

Sharding: rows of the 1024x1024 grid split across 8 cores (128 rows
each); no device collectives (circular wrap baked by host).
"""

import sys

sys.path.insert(0, "/opt/trn_rl_repo")

import ml_dtypes
import numpy as np

import concourse.bass as bass
import concourse.mybir as mybir
from concourse.bass_utils import run_bass_kernel_spmd
from concourse.tile import TileContext

H, W, C, HID = 1024, 1024, 16, 128
NCORES = 8
RPC = H // NCORES  # rows per core
CH = 512  # matmul free-dim chunk (one PSUM bank of fp32)
ZC = 3 * C  # 48 perception channels

_SOBEL_X = np.array([[-1.0, 0.0, 1.0], [-2.0, 0.0, 2.0], [-1.0, 0.0, 1.0]], np.float32)
_SOBEL_Y = np.array([[-1.0, -2.0, -1.0], [0.0, 0.0, 0.0], [1.0, 2.0, 1.0]], np.float32)

F32 = mybir.dt.float32
BF16 = mybir.dt.bfloat16
NP_BF16 = ml_dtypes.bfloat16


def _hoist_matmul_waits(nc: bass.Bass) -> None:
    """This walrus build's instruction formats hold at most ONE sync wait,
    but Tile emits 2-3 on some instructions.  Hoist excess waits onto
    inserted same-engine NoOps (one wait each) right before the
    instruction - semantically the same blocking point on the in-order
    engine queue."""
    fixn = 0
    for fn in nc.m.functions:
        for blk in fn.blocks:
            needs_fix = any(
                inst.sync_info is not None and len(inst.sync_info.on_wait) > 1
                for inst in blk.instructions
            )
            if not needs_fix:
                continue
            out = []
            for inst in blk.instructions:
                si = inst.sync_info
                if si is not None and len(si.on_wait) > 1:
                    for w in si.on_wait:
                        nop = mybir.InstNoOp(name=f"I-mmfix-{fixn}")
                        fixn += 1
                        nop.engine = inst.engine
                        nop.sync_info = mybir.SyncInfo(on_wait=[w], on_update=[])
                        out.append(nop)
                    si.on_wait = []
                out.append(inst)
            blk.instructions = out


def build_nc(rpc: int = RPC, w: int = W, hoist: bool = True) -> bass.Bass:
    ngroups = rpc // 2  # one group = 2 output rows

    nc = bass.Bass()
    zin = nc.declare_dram_parameter("zin", [2 * ZC, ngroups, w], BF16, isOutput=False)
    w1d = nc.declare_dram_parameter("w1d", [128, HID], BF16, isOutput=False)
    w2t = nc.declare_dram_parameter("w2t", [HID, 32], BF16, isOutput=False)
    b1 = nc.declare_dram_parameter("b1", [HID, 1], F32, isOutput=False)
    out = nc.declare_dram_parameter(
        "out", [128, ngroups // 4, 4 * CH], BF16, isOutput=True
    )

    with TileContext(nc) as tc:
        with (
            tc.tile_pool(name="const", bufs=1) as cpool,
            tc.tile_pool(name="xrows", bufs=6) as xpool,
            tc.tile_pool(name="hid", bufs=4) as hpool,
            tc.tile_pool(name="stage", bufs=2) as spool,
            tc.tile_pool(name="cps", bufs=1, space="PSUM") as cps,
            tc.tile_pool(name="ops", bufs=2, space="PSUM") as ops,
        ):
            # consts go on the scalar queue so the sync queue starts
            # streaming input slabs immediately
            w1d_t = cpool.tile([128, HID], BF16)
            nc.scalar.dma_start(out=w1d_t[:], in_=w1d[:])
            w2t_t = cpool.tile([HID, 32], BF16)
            nc.scalar.dma_start(out=w2t_t[:], in_=w2t[:])
            b1_t = cpool.tile([HID, 1], F32)
            nc.scalar.dma_start(out=b1_t[:], in_=b1[:])

            st_cur = {"st": None}

            def emit_mm2(hidA, hidB, g):
                stp = ops.tile([128, CH], F32, tag="o", name=f"o{g}")
                for c in range(4):
                    i, hh = divmod(c, 2)
                    hsrc = hidA if i == 0 else hidB
                    nc.tensor.matmul(
                        stp[32 * c : 32 * c + 32, :],
                        w2t_t[:, :],
                        hsrc[:, CH * hh : CH * hh + CH],
                        start=True,
                        stop=True,
                        tile_position=(0, 32 * c),
                        skip_group_check=True,
                    )
                if g % 4 == 0:
                    st_cur["st"] = spool.tile(
                        [128, 4 * CH], BF16, tag="st", name=f"st{g}"
                    )
                st = st_cur["st"]
                dst = st[:, CH * (g % 4) : CH * (g % 4) + CH]
                if g % 2 == 0:
                    nc.vector.tensor_copy(dst, stp[:])
                else:
                    nc.scalar.activation(
                        dst, stp[:], mybir.ActivationFunctionType.Copy
                    )
                if g % 4 == 3:
                    nc.gpsimd.dma_start(out=out[:, g // 4, :], in_=st[:])

            prev = None
            win_cur = {"w": None}
            for g in range(ngroups):
                # 2-group batched slab loads: even rows at partitions
                # 0-47, odd rows at 64-111 (concurrent PE row-strips)
                if g % 2 == 0:
                    winb = xpool.tile([128, 2 * w], BF16, tag="xrow", name=f"z{g}")
                    if g == 0:  # fast start: group 0 first
                        nc.sync.dma_start(out=winb[0:ZC, 0:w], in_=zin[0:ZC, 0, :])
                        nc.sync.dma_start(
                            out=winb[64 : 64 + ZC, 0:w], in_=zin[ZC : 2 * ZC, 0, :]
                        )
                        nc.sync.dma_start(out=winb[0:ZC, w : 2 * w], in_=zin[0:ZC, 1, :])
                        nc.sync.dma_start(
                            out=winb[64 : 64 + ZC, w : 2 * w],
                            in_=zin[ZC : 2 * ZC, 1, :],
                        )
                    else:
                        nc.sync.dma_start(
                            out=winb[0:ZC, :], in_=zin[0:ZC, g : g + 2, :]
                        )
                        nc.sync.dma_start(
                            out=winb[64 : 64 + ZC, :], in_=zin[ZC : 2 * ZC, g : g + 2, :]
                        )
                    win_cur["w"] = winb
                winb = win_cur["w"]
                wb = (g % 2) * w  # column base of this group inside the batch

                # conv PSUM: row-even -> 2-bank tile (DVE evac), row-odd ->
                # 2-bank tile with bufs=1 (Scalar evac, reuse distance 1)
                cvA = cps.tile([HID, 2 * CH], F32, tag="cvA", bufs=2, name=f"cva{g}")
                cvB = cps.tile([HID, 2 * CH], F32, tag="cvB", bufs=1, name=f"cvb{g}")

                # the whole 3x3x48 conv: one K=48 matmul per row per chunk,
                # both rows concurrent on disjoint PE row-strips
                for hh in range(2):
                    nc.tensor.matmul(
                        cvA[:, CH * hh : CH * hh + CH],
                        w1d_t[0:ZC, :],
                        winb[0:ZC, wb + CH * hh : wb + CH * hh + CH],
                        start=True,
                        stop=True,
                        tile_position=(0, 0),
                    )
                    nc.tensor.matmul(
                        cvB[:, CH * hh : CH * hh + CH],
                        w1d_t[64 : 64 + ZC, :],
                        winb[64 : 64 + ZC, wb + CH * hh : wb + CH * hh + CH],
                        start=True,
                        stop=True,
                        tile_position=(64, 0),
                    )

                # bias + relu evacuation, PSUM -> SBUF bf16, one fused
                # [128,1024] op per engine (DVE: row-even, Scalar: row-odd)
                hidA = hpool.tile([HID, 2 * CH], BF16, tag="hA", name=f"ha{g}")
                hidB = hpool.tile([HID, 2 * CH], BF16, tag="hB", name=f"hb{g}")
                nc.vector.tensor_scalar(
                    out=hidA[:],
                    in0=cvA[:],
                    scalar1=b1_t[:],
                    scalar2=0.0,
                    op0=mybir.AluOpType.add,
                    op1=mybir.AluOpType.max,
                )
                nc.scalar.activation(
                    hidB[:],
                    cvB[:],
                    mybir.ActivationFunctionType.Relu,
                    bias=b1_t[:],
                    scale=1.0,
                )

                # mm2 of the previous group (software pipeline keeps the PE
                # from stalling on this group's evacuation)
                if prev is not None:
                    emit_mm2(*prev)
                prev = (hidA, hidB, g)
            emit_mm2(*prev)

    if hoist:
        _hoist_matmul_waits(nc)
    return nc


_NC_CACHE: dict = {}


def _get_nc():
    if "nc" not in _NC_CACHE:
        _NC_CACHE["nc"] = build_nc()
    return _NC_CACHE["nc"]


def host_prepare(state, W1, b1, W2):
    """Build per-core input maps. state: (H, W, C) f32."""
    xt = np.ascontiguousarray(state.transpose(2, 0, 1))  # (C, H, W)
    gx = np.zeros_like(xt)
    gy = np.zeros_like(xt)
    for dy in (-1, 0, 1):
        for dx in (-1, 0, 1):
            sx = _SOBEL_X[dy + 1, dx + 1]
            sy = _SOBEL_Y[dy + 1, dx + 1]
            if sx == 0.0 and sy == 0.0:
                continue
            rolled = np.roll(xt, shift=(-dy, -dx), axis=(1, 2))
            if sx != 0.0:
                gx += sx * rolled
            if sy != 0.0:
                gy += sy * rolled
    z = np.concatenate([gx, gy, xt], axis=0).astype(NP_BF16)  # (48, H, W)

    w1d = np.zeros((128, HID), np.float32)
    w1d[0:ZC] = W1.T  # z channel order [gx, gy, x] matches W1 cols
    w1d[64 : 64 + ZC] = W1.T
    w1d = w1d.astype(NP_BF16)
    w2t32 = np.zeros((HID, 32), np.float32)
    w2t32[:, :C] = W2.T
    w2t = w2t32.astype(NP_BF16)
    b1c = np.ascontiguousarray(b1.reshape(HID, 1)).astype(np.float32)

    in_maps = []
    ngroups = RPC // 2
    for k in range(NCORES):
        zc = z[:, k * RPC : (k + 1) * RPC, :]  # (48, 128, 1024)
        s = np.empty((2 * ZC, ngroups, W), NP_BF16)
        s[0:ZC] = zc[:, 0::2, :]
        s[ZC : 2 * ZC] = zc[:, 1::2, :]
        in_maps.append(
            {
                "zin": np.ascontiguousarray(s),
                "w1d": w1d,
                "w2t": w2t,
                "b1": b1c,
            }
        )
    return in_maps


def assemble_out(results, b2):
    """results[k]["out"]: [128, RPC//8, 2048] bf16 -> full (H, W, C) f32."""
    nquads = RPC // 8
    full = np.empty((H, W, C), np.float32)
    for k in range(NCORES):
        res = np.asarray(results[k]["out"], dtype=NP_BF16).astype(np.float32)
        # partition p = 32*(2i+hh) + m (m<16 valid); free = q*CH + col
        # where group g = 4b + q
        r6 = res.reshape(2, 2, 32, nquads, 4, CH)  # [i, hh, m, b, q, col]
        valid = r6[:, :, :C]
        blk = valid.transpose(3, 4, 0, 2, 1, 5)  # [b, q, i, m, hh, col]
        blk = blk.reshape(RPC, C, W)
        full[k * RPC : (k + 1) * RPC] = blk.transpose(0, 2, 1)
    return full + b2[None, None, :].astype(np.float32)


def kernel(state, W1, b1, W2, b2, **extra):
    state = np.asarray(state, np.float32)
    W1 = np.asarray(W1, np.float32)
    b1 = np.asarray(b1, np.float32)
    W2 = np.asarray(W2, np.float32)
    b2 = np.asarray(b2, np.float32)

    nc = _get_nc()
    in_maps = host_prepare(state, W1, b1, W2)
    res = run_bass_kernel_spmd(nc, in_maps, core_ids=list(range(NCORES)))
    return np.ascontiguousarray(assemble_out(res.results, b2))


if __name__ == "__main__":
    rng = np.random.default_rng(0)
    state = rng.standard_normal((H, W, C), dtype=np.float32)
    W1 = rng.standard_normal((HID, 3 * C), dtype=np.float32) * 0.1
    b1v = rng.standard_normal(HID).astype(np.float32) * 0.1
    W2 = rng.standard_normal((C, HID), dtype=np.float32) * 0.1
    b2v = rng.standard_normal(C).astype(np.float32) * 0.1
    out = kernel(state, W1, b1v, W2, b2v)
    print(out.shape, out.dtype)
